# revision 18
# baseline (speedup 1.0000x reference)
"""Trainium2 Bass kernel for nn_BoundaryConsistencyLoss.

Math
----
Inputs seg/gt are binary {0,1} float images [64, 512, 512].  The reference
computes, per class k in {0,1}:  boundary_k = maxpool3x3(mask_k) -
minpool3x3(mask_k) (in-bounds windows), then zeroes rows whose sum >= 300,
then columns whose sum >= 300 (on the row-zeroed array), accumulates over
classes, and returns mean((pred_b - gt_b)**2).

For binary x, boundary_0 == boundary_1 == b where b[i,j] = 1 iff the 3x3
in-bounds window at (i,j) contains both a 0 and a 1.  So the loss is
4 * mean(xor(L(b_seg), L(b_gt))) with L the line-removal operator.

b is computed from a weighted window sum: replicate-pad x by 1 and take the
3x3 ones-kernel sum -> wsum in {0..9} with total weight 9 at EVERY position;
b = (wsum not in {0, 9}).

Per-sample device outputs (all exact small integers in f32):
  cs_s[j] = sum_r rowmask_s[r] * b_s[r,j]
  cs_g[j] = sum_r rowmask_g[r] * b_g[r,j]
  P[j]    = sum_r rowmask_s[r] * rowmask_g[r] * b_s[r,j] * b_g[r,j]
where rowmask = (rowsum(b) < 300).  Host finishes:
  colmask_x[j] = (cs_x[j] < 300)
  count = sum_j cs_s*colmask_s + cs_g*colmask_g - 2*P*colmask_s*colmask_g
  loss  = 4 * sum(count over samples) / (64*512*512)

Sharding: pure data parallel over batch, 8 samples per NeuronCore.
"""

from contextlib import ExitStack

import numpy as np

import concourse.bacc as bacc
import concourse.mybir as mybir
import concourse.tile as tile
from concourse import bass_utils

# ---------------------------------------------------------------- config
B, H, W = 64, 512, 512
N_CORES = 8
BPC = B // N_CORES  # samples per core

LINE_T = 300.0

# Feature flags (fallbacks if a fast path misbehaves on HW).
CAST_DMA = True      # SWDGE cast-DMA f32->bf16 for matmul inputs (full-rate PE)
USE_MOD = False      # mod is not a valid HW STT op; use the 2-pass threshold
GPSIMD_PASS = "w1"   # "", "w1": offload one horizontal add pass to GpSimd

# Row tiling: (input_row_lo, K=input_rows, valid_out_rows, global_out_lo)
TILES = [
    (0, 128, 126, 0),
    (125, 128, 126, 126),
    (251, 128, 126, 252),
    (377, 128, 126, 378),
    (503, 9, 8, 504),
]
NT = len(TILES)


def _build_bmat() -> np.ndarray:
    """Vertical band matrices, bmat[k, t*128 + m] = weight of input row k of
    tile t for output row m.  Includes replicate-pad edge doubling."""
    bm = np.zeros((128, NT * 128), np.float32)
    for t, (lo, K, mv, glo) in enumerate(TILES):
        for m in range(mv):
            g = glo + m  # global output row
            for gk in (g - 1, g, g + 1):
                gk_c = min(max(gk, 0), H - 1)  # replicate pad
                k = gk_c - lo
                assert 0 <= k < K, (t, m, gk_c, k)
                bm[k, t * 128 + m] += 1.0
    return bm


def _build_module(bpc: int = BPC):
    nc = bacc.Bacc("TRN2")
    f32 = mybir.dt.float32
    bf16 = mybir.dt.bfloat16
    f32r = mybir.dt.float32r
    Alu = mybir.AluOpType

    seg = nc.dram_tensor("seg", [bpc, H, W], f32, kind="ExternalInput")
    gt = nc.dram_tensor("gt", [bpc, H, W], f32, kind="ExternalInput")
    bmat = nc.dram_tensor("bmat", [128, NT * 128], f32, kind="ExternalInput")
    # 3*bpc result vectors of 512 (cs_s, cs_g, P per sample), on one partition
    out = nc.dram_tensor("out", [1, 3 * bpc * W], f32, kind="ExternalOutput")

    with tile.TileContext(nc) as tc, ExitStack() as ctx:
        const = ctx.enter_context(tc.tile_pool(name="const", bufs=1))
        xp = ctx.enter_context(tc.tile_pool(name="xp", bufs=6))
        sp = ctx.enter_context(tc.tile_pool(name="sp", bufs=4))
        wp = ctx.enter_context(tc.tile_pool(name="wp", bufs=8))
        bp = ctx.enter_context(tc.tile_pool(name="bp", bufs=4))
        qp = ctx.enter_context(tc.tile_pool(name="qp", bufs=2))
        sm = ctx.enter_context(tc.tile_pool(name="sm", bufs=12))
        rp = ctx.enter_context(tc.tile_pool(name="rp", bufs=1))
        pv = ctx.enter_context(tc.tile_pool(name="pv", bufs=2, space="PSUM"))
        pa = ctx.enter_context(tc.tile_pool(name="pa", bufs=1, space="PSUM"))

        mm_dt = bf16 if CAST_DMA else f32
        mm_dma = nc.gpsimd if CAST_DMA else nc.sync
        Bs = const.tile([128, NT * 128], mm_dt)
        mm_dma.dma_start(out=Bs[:], in_=bmat[:])
        ones = const.tile([128, W], bf16)
        nc.vector.memset(ones[:], 1.0)

        # 6 PSUM banks of accumulators; slot i=3*s+k uses bank i%6 (at most
        # one open accumulation group per bank at any time, groups of
        # consecutive samples land on disjoint banks).
        acc = pa.tile([128, 6 * 512], f32)
        res = rp.tile([1, 3 * bpc * W], f32)

        def slot_ap(i):
            c = i % 6
            return acc[0:1, 512 * c : 512 * (c + 1)]

        def process_tensor(x_dram, s, t):
            """One (sample, tile) stage for one input tensor.
            Returns (b, m_rep) SBUF tiles."""
            lo, K, mv, glo = TILES[t]
            X = xp.tile([128, W], mm_dt, tag="x")
            mm_dma.dma_start(out=X[:K, :], in_=x_dram[s, lo : lo + K, :])

            # vertical band sum -> PSUM, values in {0..3}
            v = pv.tile([128, W], f32, tag="v")
            nc.tensor.matmul(
                v[:, :], Bs[:K, t * 128 : (t + 1) * 128], X[:K, :],
                start=True, stop=True,
            )

            # evacuate PSUM -> SBUF bf16, into S[:, 1:513]
            S = sp.tile([128, W + 1], bf16, tag="s")
            nc.scalar.copy(S[:, 1 : W + 1], v[:, :])
            # S[:,0] := s[0] (replicate pad left)
            nc.vector.tensor_copy(S[:, 0:1], S[:, 1:2])

            # horizontal 3-tap sum with replicate pad:
            #   W1[j] = s[j] + s[j+1] (j<511),  W1[511] = 2*s[511]
            #   Wt[j] = s[j-1] + W1[j]  (s[-1] := s[0] via S[:,0])
            W1 = wp.tile([128, W], bf16, tag="w1")
            w1_eng = nc.gpsimd if GPSIMD_PASS == "w1" else nc.vector
            w1_eng.tensor_tensor(W1[:, 0 : W - 1], S[:, 1:W], S[:, 2 : W + 1], Alu.add)
            nc.vector.tensor_scalar(
                W1[:, W - 1 : W], S[:, W : W + 1], 2.0, None, Alu.mult
            )
            Wt = wp.tile([128, W], bf16, tag="w")
            nc.vector.tensor_tensor(Wt[:, :], S[:, 0:W], W1[:, :], Alu.add)

            # threshold: b = 1 iff wsum not in {0, 9}; rowsum via accum_out
            b = bp.tile([128, W], bf16, tag="b")
            rs = sm.tile([128, 1], f32, tag="rs")
            if USE_MOD:
                # b = ((Wt mod 9) >= 1); accum_out is always sum(out)
                nc.vector.scalar_tensor_tensor(
                    b[:, :], Wt[:, :], 9.0, ones[:, :], Alu.mod, Alu.is_ge,
                    accum_out=rs[:],
                )
            else:
                u = wp.tile([128, W], bf16, tag="u")
                nc.vector.tensor_scalar(u[:, :], Wt[:, :], 1.0, None, Alu.is_ge)
                nc.vector.scalar_tensor_tensor(
                    b[:, :], Wt[:, :], 9.0, u[:, :], Alu.is_lt, Alu.mult,
                    accum_out=rs[:],
                )

            # row keep-mask (colsum matmul lhsT, M=1)
            m = sm.tile([128, 1], bf16, tag="m")
            nc.vector.tensor_scalar(m[:], rs[:], LINE_T, None, Alu.is_lt)
            return b, m

        def evac(i):
            """After slot i's group closed, copy [1,512] PSUM -> res slot."""
            nc.scalar.copy(res[0:1, 512 * i : 512 * (i + 1)], slot_ap(i))

        for s in range(bpc):
            for t in range(NT):
                b_s, m_s = process_tensor(seg, s, t)
                b_g, m_g = process_tensor(gt, s, t)

                start, stop = (t == 0), (t == NT - 1)
                nc.tensor.matmul(slot_ap(3 * s + 0), m_s[:], b_s[:],
                                 start=start, stop=stop)
                nc.tensor.matmul(slot_ap(3 * s + 1), m_g[:], b_g[:],
                                 start=start, stop=stop)

                q = qp.tile([128, W], bf16, tag="q")
                nc.vector.tensor_tensor(q[:, :], b_s[:, :], b_g[:, :], Alu.mult)
                mq = sm.tile([128, 1], bf16, tag="mq")
                nc.vector.tensor_tensor(mq[:], m_s[:], m_g[:], Alu.mult)
                nc.tensor.matmul(slot_ap(3 * s + 2), mq[:], q[:],
                                 start=start, stop=stop)
            for k in range(3):
                evac(3 * s + k)

        nc.sync.dma_start(out=out[:], in_=res[:])

    nc.compile()
    return nc


_CACHE: dict = {}


def _get_module():
    if "nc" not in _CACHE:
        _CACHE["nc"] = _build_module()
        _CACHE["bmat"] = _build_bmat()
    return _CACHE["nc"], _CACHE["bmat"]


def _host_finish(res_per_core: list[np.ndarray]) -> np.ndarray:
    """res arrays are [1, 3*BPC*512] f32; slot i=3*s+k at [0, 512*i:512*(i+1)]."""
    total = 0.0
    for res in res_per_core:
        for s in range(BPC):
            vecs = []
            for k in range(3):
                i = 3 * s + k
                vecs.append(res[0, 512 * i : 512 * (i + 1)].astype(np.float64))
            cs_s, cs_g, P = vecs
            ok_s = (cs_s < LINE_T).astype(np.float64)
            ok_g = (cs_g < LINE_T).astype(np.float64)
            total += float(
                np.sum(cs_s * ok_s) + np.sum(cs_g * ok_g) - 2.0 * np.sum(P * ok_s * ok_g)
            )
    return np.asarray(np.float32(4.0 * total / float(B * H * W)))


def kernel(seg: np.ndarray, gt: np.ndarray) -> np.ndarray:
    nc, bm = _get_module()
    seg = np.ascontiguousarray(seg, dtype=np.float32)
    gt = np.ascontiguousarray(gt, dtype=np.float32)
    in_maps = [
        {
            "seg": seg[c * BPC : (c + 1) * BPC],
            "gt": gt[c * BPC : (c + 1) * BPC],
            "bmat": bm,
        }
        for c in range(N_CORES)
    ]
    r = bass_utils.run_bass_kernel_spmd(nc, in_maps, core_ids=list(range(N_CORES)))
    return _host_finish([r.results[c]["out"] for c in range(N_CORES)])


# revision 23
# speedup vs baseline: 1.4956x; 1.4956x over previous
"""Trainium2 Bass kernel for nn_BoundaryConsistencyLoss.

Math
----
Inputs seg/gt are binary {0,1} float images [64, 512, 512].  For binary x the
per-class boundary (dilation - erosion, in-bounds 3x3 windows) is identical
for both classes:  b[i,j] = 1 iff the 3x3 in-bounds window at (i,j) is
non-constant.  The loss reduces to 4 * mean(xor(L(b_seg), L(b_gt))) with L
the row/column line-removal operator.

b is computed from a weighted window sum: replicate-pad x by 1 and take the
3x3 ones-kernel sum -> wsum in {0..9} with total weight 9 at EVERY position;
b = (wsum not in {0, 9}) = (|wsum - 4.5| < 4.5).

Per-sample device outputs (exact small integers in f32):
  cs_s[j] = sum_r rowmask_s[r] * b_s[r,j]
  cs_g[j] = sum_r rowmask_g[r] * b_g[r,j]
  P[j]    = sum_r rowmask_s[r] * rowmask_g[r] * b_s[r,j] * b_g[r,j]
where rowmask = (rowsum(b) < 300).  Host finishes with the column masks and
the xor-count identity (exact in f64).

Sharding: pure data parallel over batch, 8 samples per NeuronCore.

Device pipeline per (sample, tensor, row-tile):
  - DMA bf16 tile (host pre-casts f32->bf16; exact for {0,1}).
  - horizontal 3-tap sum: on PE via 3 PSUM-accumulated matmuls with shifted
    rhs (tiles 0-2), or on DVE via 2 adds (tiles 3-4) - engine balancing.
  - vertical 3-tap sum via a banded matmul (B matrices baked on host).
  - ScalarE evacuates PSUM with Abs: a = |wsum - 4.5| (bf16, exact).
  - VectorE: b = (a < 4.5) with fused accum -> rowsum; tiny row mask.
  - TensorE: masked column sums (cs, and P from q = b_s*b_g) accumulated in
    PSUM banks; evacuated per sample to SBUF; one DMA out at the end.
"""

from contextlib import ExitStack

import ml_dtypes
import numpy as np

import concourse.bacc as bacc
import concourse.mybir as mybir
import concourse.tile as tile
from concourse import bass_utils

# ---------------------------------------------------------------- config
B, H, W = 64, 512, 512
N_CORES = 8
BPC = B // N_CORES  # samples per core

LINE_T = 300.0

# tiles 0..PE_H_TILES-1 do the horizontal conv on PE, the rest on DVE
PE_H_TILES = 3

# Row tiling: (input_row_lo, K=input_rows, valid_out_rows, global_out_lo)
TILES = [
    (0, 128, 126, 0),
    (125, 128, 126, 126),
    (251, 128, 126, 252),
    (377, 128, 126, 378),
    (503, 9, 8, 504),
]
NT = len(TILES)


def _build_bmat() -> np.ndarray:
    """Vertical band matrices, bmat[k, t*128 + m] = weight of input row k of
    tile t for output row m.  Includes replicate-pad edge doubling."""
    bm = np.zeros((128, NT * 128), np.float32)
    for t, (lo, K, mv, glo) in enumerate(TILES):
        for m in range(mv):
            g = glo + m  # global output row
            for gk in (g - 1, g, g + 1):
                gk_c = min(max(gk, 0), H - 1)  # replicate pad
                k = gk_c - lo
                assert 0 <= k < K, (t, m, gk_c, k)
                bm[k, t * 128 + m] += 1.0
    return bm.astype(ml_dtypes.bfloat16)


def _build_module(bpc: int = BPC):
    nc = bacc.Bacc("TRN2")
    f32 = mybir.dt.float32
    bf16 = mybir.dt.bfloat16
    Alu = mybir.AluOpType

    seg = nc.dram_tensor("seg", [bpc, H, W], bf16, kind="ExternalInput")
    gt = nc.dram_tensor("gt", [bpc, H, W], bf16, kind="ExternalInput")
    bmat = nc.dram_tensor("bmat", [128, NT * 128], bf16, kind="ExternalInput")
    # 3*bpc result vectors of 512 (cs_s, cs_g, P per sample), on one partition
    out = nc.dram_tensor("out", [1, 3 * bpc * W], f32, kind="ExternalOutput")

    with tile.TileContext(nc) as tc, ExitStack() as ctx:
        const = ctx.enter_context(tc.tile_pool(name="const", bufs=1))
        xp = ctx.enter_context(tc.tile_pool(name="xp", bufs=6))
        hp = ctx.enter_context(tc.tile_pool(name="hp", bufs=4))
        ap_ = ctx.enter_context(tc.tile_pool(name="ap", bufs=4))
        bp = ctx.enter_context(tc.tile_pool(name="bp", bufs=4))
        qp = ctx.enter_context(tc.tile_pool(name="qp", bufs=2))
        sm = ctx.enter_context(tc.tile_pool(name="sm", bufs=12))
        rp = ctx.enter_context(tc.tile_pool(name="rp", bufs=1))
        pv = ctx.enter_context(tc.tile_pool(name="pv", bufs=3, space="PSUM"))
        pa = ctx.enter_context(tc.tile_pool(name="pa", bufs=1, space="PSUM"))

        Bs = const.tile([128, NT * 128], bf16)
        nc.sync.dma_start(out=Bs[:], in_=bmat[:])
        nbias = const.tile([128, 1], f32)
        nc.vector.memset(nbias[:], -4.5)

        # 5 PSUM banks of accumulators (pv uses 3); slot i=3*s+k -> bank i%5.
        acc = pa.tile([128, 5 * 512], f32)
        res = rp.tile([1, 3 * bpc * W], f32)

        def slot_ap(i):
            c = i % 5
            return acc[0:1, 512 * c : 512 * (c + 1)]

        def evac(i, on_act):
            """After slot i's group closed, copy [1,512] PSUM -> res slot."""
            dst = res[0:1, 512 * i : 512 * (i + 1)]
            if on_act:
                nc.scalar.copy(dst, slot_ap(i))
            else:
                nc.vector.tensor_copy(dst, slot_ap(i))

        def process_tensor(x_dram, s, t):
            """One (sample, tile) stage for one input tensor.
            Returns (b, m) SBUF tiles."""
            lo, K, mv, glo = TILES[t]
            X = xp.tile([128, W], bf16, tag="x")
            nc.sync.dma_start(out=X[:K, :], in_=x_dram[s, lo : lo + K, :])

            Bt = Bs[:K, t * 128 : (t + 1) * 128]
            ps = pv.tile([128, W], f32, tag="v")
            if t < PE_H_TILES:
                # wsum via 3 PSUM-accumulated matmuls with shifted rhs
                # (+ 2 tiny edge-doubling matmuls).
                nc.tensor.matmul(ps[:, 0:W], Bt, X[:K, 0:W],
                                 start=True, stop=False)
                nc.tensor.matmul(ps[:, 1:W], Bt, X[:K, 0 : W - 1],
                                 start=False, stop=False)
                nc.tensor.matmul(ps[:, 0 : W - 1], Bt, X[:K, 1:W],
                                 start=False, stop=False)
                nc.tensor.matmul(ps[:, 0:1], Bt, X[:K, 0:1],
                                 start=False, stop=False)
                nc.tensor.matmul(ps[:, W - 1 : W], Bt, X[:K, W - 1 : W],
                                 start=False, stop=True)
            else:
                # horizontal 3-tap on DVE into hX (offset by 1 col to keep
                # the second add 4B-aligned), then one banded matmul.
                # w1[j] = x[j] + x[j+1]          (j = 0..W-2)
                # hX[1+j] = w1[j-1] + x[j+1] = x[j-1]+x[j]+x[j+1]
                w1 = hp.tile([128, W], bf16, tag="w1")
                nc.vector.tensor_tensor(
                    w1[:K, 0 : W - 1], X[:K, 0 : W - 1], X[:K, 1:W], Alu.add
                )
                hX = hp.tile([128, W + 2], bf16, tag="hx")
                nc.vector.tensor_tensor(
                    hX[:K, 2:W], w1[:K, 0 : W - 2], X[:K, 2:W], Alu.add
                )
                # edges (replicate-pad doubling)
                nc.vector.tensor_tensor(
                    hX[:K, 1:2], w1[:K, 0:1], X[:K, 0:1], Alu.add
                )
                nc.vector.tensor_tensor(
                    hX[:K, W : W + 1], w1[:K, W - 2 : W - 1], X[:K, W - 1 : W],
                    Alu.add,
                )
                nc.tensor.matmul(ps[:, :], Bt, hX[:K, 1 : W + 1],
                                 start=True, stop=True)

            # evacuate PSUM with Abs: a = |wsum - 4.5| in {0.5..4.5}, bf16
            a = ap_.tile([128, W], bf16, tag="a")
            nc.scalar.activation(
                a[:, :], ps[:, :], mybir.ActivationFunctionType.Abs, bias=nbias[:]
            )

            # b = (a < 4.5); accum_out (op1=add) -> rowsum
            b = bp.tile([128, W], bf16, tag="b")
            rs = sm.tile([128, 1], f32, tag="rs")
            nc.vector.tensor_scalar(
                b[:, :], a[:, :], 4.5, None, Alu.is_lt, Alu.add, accum_out=rs[:]
            )

            # row keep-mask (colsum matmul lhsT, M=1)
            m = sm.tile([128, 1], bf16, tag="m")
            nc.vector.tensor_scalar(m[:], rs[:], LINE_T, None, Alu.is_lt)
            return b, m

        for s in range(bpc):
            for t in range(NT):
                b_s, m_s = process_tensor(seg, s, t)
                b_g, m_g = process_tensor(gt, s, t)

                start, stop = (t == 0), (t == NT - 1)
                nc.tensor.matmul(slot_ap(3 * s + 0), m_s[:], b_s[:],
                                 start=start, stop=stop)
                nc.tensor.matmul(slot_ap(3 * s + 1), m_g[:], b_g[:],
                                 start=start, stop=stop)

                q = qp.tile([128, W], bf16, tag="q")
                nc.vector.tensor_tensor(q[:, :], b_s[:, :], b_g[:, :], Alu.mult)
                mq = sm.tile([128, 1], bf16, tag="mq")
                nc.vector.tensor_tensor(mq[:], m_s[:], m_g[:], Alu.mult)
                nc.tensor.matmul(slot_ap(3 * s + 2), mq[:], q[:],
                                 start=start, stop=stop)
            # evacuate this sample's three vectors (split ACT/DVE)
            evac(3 * s + 0, on_act=True)
            evac(3 * s + 1, on_act=True)
            evac(3 * s + 2, on_act=False)

        nc.sync.dma_start(out=out[:], in_=res[:])

    nc.compile()
    return nc


_CACHE: dict = {}


def _get_module():
    if "nc" not in _CACHE:
        _CACHE["nc"] = _build_module()
        _CACHE["bmat"] = _build_bmat()
    return _CACHE["nc"], _CACHE["bmat"]


def _host_finish(res_per_core: list[np.ndarray]) -> np.ndarray:
    """res arrays are [1, 3*BPC*512] f32; slot i=3*s+k at [0, 512*i:512*(i+1)]."""
    total = 0.0
    for res in res_per_core:
        for s in range(BPC):
            vecs = []
            for k in range(3):
                i = 3 * s + k
                vecs.append(res[0, 512 * i : 512 * (i + 1)].astype(np.float64))
            cs_s, cs_g, P = vecs
            ok_s = (cs_s < LINE_T).astype(np.float64)
            ok_g = (cs_g < LINE_T).astype(np.float64)
            total += float(
                np.sum(cs_s * ok_s) + np.sum(cs_g * ok_g) - 2.0 * np.sum(P * ok_s * ok_g)
            )
    return np.asarray(np.float32(4.0 * total / float(B * H * W)))


def kernel(seg: np.ndarray, gt: np.ndarray) -> np.ndarray:
    nc, bm = _get_module()
    seg = np.ascontiguousarray(seg, dtype=np.float32).astype(ml_dtypes.bfloat16)
    gt = np.ascontiguousarray(gt, dtype=np.float32).astype(ml_dtypes.bfloat16)
    in_maps = [
        {
            "seg": seg[c * BPC : (c + 1) * BPC],
            "gt": gt[c * BPC : (c + 1) * BPC],
            "bmat": bm,
        }
        for c in range(N_CORES)
    ]
    r = bass_utils.run_bass_kernel_spmd(nc, in_maps, core_ids=list(range(N_CORES)))
    return _host_finish([r.results[c]["out"] for c in range(N_CORES)])
